# revision 6
# baseline (speedup 1.0000x reference)
"""Trainium2 Bass kernel for BuiltSWAP: out = (state_re + i*state_im) @ M.

M is in practice the SWAP(0,7)-gate permutation matrix on 13 qubits: the
whole matmul is mathematically a column permutation of state that swaps bit
12 and bit 5 of the column index (out[:, j] = state[:, j ^ 4128] when those
bits differ).  The fast path exploits this: no matmul at all, just a data
movement kernel.

Fast path (verified on host: M must be exactly that permutation matrix):
  - Data-parallel shard: core c handles batch rows 8c..8c+8 of re and im
    (16 rows x 8192 f32 = 512 KB per core in, 512 KB out).
  - SBUF partition index = (row, col bits 11..9) -- bits the permutation
    does NOT touch -- so both DMAs move 4 KB-contiguous descriptors at full
    DMA-bus speed, and the bit12<->bit5 swap becomes a pure within-partition
    strided copy: out_sb[p, x, m, y, l] = in_sb[p, y, m, x, l]
    (free dims x=bit12, m=bits 8..6, y=bit5, l=bits 4..0; host pre-permutes
    bits 11..9 into the partition index).
  - The shuffle is split across DVE and ACT (one x-half each, ~0.5us), fully
    hidden under the DMAs (~2.9us/rep at the 360 GB/s DMA-bus roofline).

Fallback for an unexpected M: dense matmul path (column-sharded tensor
parallelism, fp16 hi/lo split state x fp8 M, ~55-60us) -- see
_build_matmul_program below.
"""

import numpy as np
import ml_dtypes

BATCH = 64
N = 8192
NCORES = 8
COLS = N // NCORES          # 1024 output columns per core
P = 128                     # partitions
KT = N // P                 # 64 k-tiles
NCH = COLS // 512           # 2 psum chunks of 512
KBLK = 8                    # max k-tiles per M DMA block
# progressive DMA block schedule: small first blocks let the first matmuls
# start ~3us earlier (measured win on both single-shot and steady state)
BLOCKS = [2, 2, 4] + [8] * 7
NBLK = len(BLOCKS)

f8e4 = ml_dtypes.float8_e4m3
SCALE_BITS = 22
SCALE = float(2 ** SCALE_BITS)
INV_SCALE = float(2.0 ** (-SCALE_BITS))

_cached = {}

# --- permutation fast path ---------------------------------------------------
SWAP_MASK = (1 << 12) | (1 << 5)  # 4128: SWAP(0,7) on 13 qubits, bit-flipped
ROWS = 2 * BATCH // NCORES        # 16 rows per core (8 re + 8 im)


def _is_expected_perm(M):
    """True iff M is exactly the bit12<->bit5 column-swap permutation."""
    if M.shape != (N, N):
        return False
    idx = np.arange(N)
    differ = ((idx >> 12) & 1) != ((idx >> 5) & 1)
    swp = np.where(differ, idx ^ SWAP_MASK, idx)
    if not np.all(M[idx, swp] == 1.0):
        return False
    # the 8192 checked entries are exactly 1; 8192 nonzeros total => all
    # other entries are exactly 0, i.e. M is exactly this permutation
    return np.count_nonzero(M) == N


def _build_permute_program(loop_n=None, unroll=1):
    """Permutation kernel: out[p, x, m, y, l] = x[p, y, m, x, l].

    DRAM/SBUF layout [128, 2, 8, 2, 32] f32: p = (row, col bits 11..9)
    (host pre-permuted), then free dims x=bit12, m=bits 8..6, y=bit5,
    l=bits 4..0.  loop_n!=None wraps `unroll` reps in a hardware For_i loop
    for slope timing (one NEFF, loop_n iterations).
    """
    import concourse.mybir as mybir
    import concourse.tile as tile
    from concourse import bacc

    f32 = mybir.dt.float32
    nc = bacc.Bacc("TRN2", target_bir_lowering=False, debug=False)
    x_d = nc.declare_dram_parameter("x", [128, 2, 8, 2, 32], f32, isOutput=False)
    out_d = nc.declare_dram_parameter("out", [128, 2, 8, 2, 32], f32, isOutput=True)

    with tile.TileContext(nc) as tc:
        with tc.tile_pool(name="io", bufs=2) as iop:

            def rep():
                in_sb = iop.tile([128, 2, 8, 2, 32], f32, name="in_sb")
                out_sb = iop.tile([128, 2, 8, 2, 32], f32, name="out_sb")
                nc.sync.dma_start(in_sb[:], x_d[:])
                # swap bit12 (x) and bit5 (y): one x-half per engine so the
                # two copies run concurrently on DVE and ACT
                nc.vector.tensor_copy(
                    out_sb[:, 0], in_sb[:, :, :, 0, :].transpose([0, 2, 1, 3])
                )
                nc.scalar.copy(
                    out_sb[:, 1], in_sb[:, :, :, 1, :].transpose([0, 2, 1, 3])
                )
                nc.scalar.dma_start(out_d[:], out_sb[:])

            if loop_n is None:
                for _ in range(unroll):
                    rep()
            else:
                with tc.For_i(0, loop_n):
                    for _ in range(unroll):
                        rep()
    nc.compile()
    return nc


def _prep_perm_inputs(state_re, state_im):
    """Per-core [128, 2, 8, 2, 32] arrays: p=(row, col bits 11..9), then
    (bit12, bits 8..6, bit5, bits 4..0)."""
    maps = []
    rpc = ROWS // 2  # 8 batch rows per core
    for c in range(NCORES):
        rows = np.concatenate(
            [state_re[c * rpc:(c + 1) * rpc], state_im[c * rpc:(c + 1) * rpc]],
            axis=0,
        )  # [16, 8192]
        v = rows.reshape(ROWS, 2, 8, 512).transpose(0, 2, 1, 3)  # r, m53, x, rest
        maps.append({"x": np.ascontiguousarray(v).reshape(128, 2, 8, 2, 32)})
    return maps


def _post_perm(results):
    re_parts, im_parts = [], []
    rpc = ROWS // 2
    for c in range(NCORES):
        o = np.asarray(results[c]["out"]).reshape(ROWS, 8, 2, 512)
        o = o.transpose(0, 2, 1, 3).reshape(ROWS, N)
        re_parts.append(o[:rpc])
        im_parts.append(o[rpc:])
    out_re = np.concatenate(re_parts, axis=0)
    out_im = np.concatenate(im_parts, axis=0)
    return (out_re + 1j * out_im).astype(np.complex64)


# --- dense matmul fallback ---------------------------------------------------
def _fp8_exact(M):
    # cheap exactness check: fp8e4m3 round-trips M losslessly?
    sample = M[:: 64, :: 64]
    if not np.array_equal(sample.astype(f8e4).astype(np.float32), sample):
        return False
    return np.array_equal(M.astype(f8e4).astype(np.float32), M)


def _build_matmul_program(reps=1, serialize=False, m_dt="fp8"):
    # reps>1 repeats the whole pipeline inside one NEFF (for benchmarking);
    # serialize adds an all-engine barrier between reps so the per-rep slope
    # approximates a single-shot kernel execution.
    import concourse.mybir as mybir
    import concourse.tile as tile
    from concourse import bacc

    mdt = {"fp8": mybir.dt.float8e4, "bf16": mybir.dt.bfloat16}[m_dt]
    nc = bacc.Bacc("TRN2", target_bir_lowering=False, debug=False)
    st_d = nc.declare_dram_parameter("st", [P, KT, 256], mybir.dt.float16, isOutput=False)
    m_d = nc.declare_dram_parameter("m", [P, KT, NCH, 512], mdt, isOutput=False)
    out_d = nc.declare_dram_parameter("out", [P, COLS], mybir.dt.float32, isOutput=True)

    with tile.TileContext(nc) as tc:
        with (
            tc.tile_pool(name="stp", bufs=1) as stp,
            tc.tile_pool(name="mp", bufs=4) as mp,
            tc.tile_pool(name="op", bufs=1) as op,
            tc.tile_pool(name="ps", bufs=1, space="PSUM") as ps,
        ):
            st_sb = stp.tile([P, KT, 256], mybir.dt.float16)
            # split the state load so the first matmuls aren't gated on 4MB
            k0 = 0
            for nb in BLOCKS:
                nc.sync.dma_start(st_sb[:, k0:k0 + nb, :], st_d[:, k0:k0 + nb, :])
                k0 += nb
            # dummy matmuls on a zeroed scratch tile run during the initial
            # DMA wait and release the PE HAM clock throttle (1.2 -> 2.4 GHz)
            # before the real matmuls start (measured ~5us single-shot win)
            wsb = stp.tile([P, 128], mybir.dt.float16, name="wsb")
            nc.vector.memset(wsb[:], 0.0)
            wps = ps.tile([P, 128], mybir.dt.float32, name="wps")
            for _rep in range(reps):
                if serialize and reps > 1:
                    tc.strict_bb_all_engine_barrier()
                for _ in range(40):
                    nc.tensor.matmul(wps[:], wsb[:], wsb[:], start=True, stop=True)
                out_sb = op.tile([P, COLS], mybir.dt.float32, name="out_sb")
                ps_hi = [
                    ps.tile([P, 512], mybir.dt.float32, name=f"ps_hi{i}")
                    for i in range(NCH)
                ]
                ps_lo = [
                    ps.tile([P, 512], mybir.dt.float32, name=f"ps_lo{i}")
                    for i in range(NCH)
                ]
                k0 = 0
                for nb in BLOCKS:
                    m_sb = mp.tile([P, KBLK, NCH, 512], mdt, name="m_sb")
                    nc.sync.dma_start(m_sb[:, :nb], m_d[:, k0:k0 + nb, :, :])
                    for kj in range(nb):
                        ko = k0 + kj
                        # pass-major order: the stationary operand (hi or lo
                        # state tile) is reused across both n-chunks, halving
                        # LDWEIGHTS traffic vs alternating hi/lo per chunk
                        for pss, c0 in ((ps_hi, 0), (ps_lo, 128)):
                            for nch in range(NCH):
                                nc.tensor.matmul(
                                    pss[nch][:],
                                    st_sb[:, ko, c0:c0 + 128],
                                    m_sb[:, kj, nch, :],
                                    start=(ko == 0),
                                    stop=(ko == KT - 1),
                                )
                    k0 += nb
                for nch in range(NCH):
                    sl = slice(nch * 512, (nch + 1) * 512)
                    nc.vector.tensor_scalar_mul(out_sb[:, sl], ps_lo[nch][:], INV_SCALE)
                    nc.vector.tensor_add(out_sb[:, sl], out_sb[:, sl], ps_hi[nch][:])
                nc.sync.dma_start(out_d[:], out_sb[:])
    nc.compile()
    return nc


def _get_program(key, builder, **kw):
    if key not in _cached:
        _cached[key] = builder(**kw)
    return _cached[key]


def _prep_inputs(state_re, state_im, M, m_dt="fp8"):
    # Stationary layout: [8192, 256] fp16 where cols 0:64 re_hi, 64:128 im_hi,
    # 128:192 re_lo*2^22, 192:256 im_lo*2^22; tiled to [128 part, 64 kt, 256].
    S = np.empty((N, P), dtype=np.float32)
    S[:, :BATCH] = state_re.T
    S[:, BATCH:] = state_im.T
    hi = S.astype(np.float16)
    lo = ((S - hi.astype(np.float32)) * SCALE).astype(np.float16)
    stall = np.concatenate([hi, lo], axis=1)  # [8192, 256] fp16
    st_tiled = np.ascontiguousarray(
        stall.reshape(KT, P, 256).transpose(1, 0, 2)
    )  # [128, 64, 256]

    Mb = M.astype(f8e4 if m_dt == "fp8" else ml_dtypes.bfloat16)
    m_tiles = []
    for c in range(NCORES):
        shard = Mb[:, c * COLS:(c + 1) * COLS]
        m_tiles.append(
            np.ascontiguousarray(
                shard.reshape(KT, P, NCH, 512).transpose(1, 0, 2, 3)
            )
        )  # [128, 64, 2, 512]
    return st_tiled, m_tiles


def run_on_hw(state_re, state_im, M, trace=False):
    from concourse.bass_utils import run_bass_kernel_spmd

    state_re = np.asarray(state_re, dtype=np.float32)
    state_im = np.asarray(state_im, dtype=np.float32)
    M = np.asarray(M, dtype=np.float32)

    if state_re.shape == (BATCH, N) and _is_expected_perm(M):
        # fast path: M is exactly the SWAP permutation -> pure data movement
        nc = _get_program("perm", _build_permute_program)
        in_maps = _prep_perm_inputs(state_re, state_im)
        res = run_bass_kernel_spmd(
            nc, in_maps, list(range(NCORES)), trace=trace,
            trace_cores=list(range(NCORES)) if trace else None,
        )
        return _post_perm(res.results), res

    # fallback: dense matmul.  fp8e4m3 storage of M is exact only for values
    # with <=4 significand bits; fall back to bf16 if fp8 would round.
    m_dt = "fp8" if _fp8_exact(M) else "bf16"
    nc = _get_program(f"nc_{m_dt}", _build_matmul_program, m_dt=m_dt)
    st_tiled, m_tiles = _prep_inputs(state_re, state_im, M, m_dt)
    in_maps = [{"st": st_tiled, "m": m_tiles[c]} for c in range(NCORES)]
    res = run_bass_kernel_spmd(
        nc, in_maps, list(range(NCORES)), trace=trace,
        trace_cores=list(range(NCORES)) if trace else None,
    )
    full = np.concatenate([res.results[c]["out"] for c in range(NCORES)], axis=1)
    out = (full[:BATCH] + 1j * full[BATCH:]).astype(np.complex64)
    return out, res


def kernel(state_re, state_im, M):
    out, _ = run_on_hw(state_re, state_im, M, trace=False)
    return out



# revision 7
# speedup vs baseline: 1.4475x; 1.4475x over previous
"""Trainium2 Bass kernel for BuiltSWAP: out = (state_re + i*state_im) @ M.

M is in practice the SWAP(0,7)-gate permutation matrix on 13 qubits: the
whole matmul is mathematically a column permutation of state that swaps bit
12 and bit 5 of the column index (out[:, j] = state[:, j ^ 4128] when those
bits differ).  The fast path exploits this: no matmul at all, just a data
movement kernel.

Fast path (verified on host: M must be exactly that permutation matrix):
  - Data-parallel shard: core c handles batch rows 8c..8c+8 of re and im
    (16 rows x 8192 f32 = 512 KB per core in, 512 KB out).
  - SBUF partition index = (row, col bits 11..9) -- bits the permutation
    does NOT touch -- so both DMAs move 4 KB-contiguous descriptors at full
    DMA-bus speed, and the bit12<->bit5 swap becomes a pure within-partition
    strided copy: out_sb[p, x, m, y, l] = in_sb[p, y, m, x, l]
    (free dims x=bit12, m=bits 8..6, y=bit5, l=bits 4..0; host pre-permutes
    bits 11..9 into the partition index).
  - The shuffle is split across DVE and ACT (one x-half each, ~0.5us), fully
    hidden under the DMAs (~2.9us/rep at the 360 GB/s DMA-bus roofline).

Fallback for an unexpected M: dense matmul path (column-sharded tensor
parallelism, fp16 hi/lo split state x fp8 M, ~55-60us) -- see
_build_matmul_program below.
"""

import numpy as np
import ml_dtypes

BATCH = 64
N = 8192
NCORES = 8
COLS = N // NCORES          # 1024 output columns per core
P = 128                     # partitions
KT = N // P                 # 64 k-tiles
NCH = COLS // 512           # 2 psum chunks of 512
KBLK = 8                    # max k-tiles per M DMA block
# progressive DMA block schedule: small first blocks let the first matmuls
# start ~3us earlier (measured win on both single-shot and steady state)
BLOCKS = [2, 2, 4] + [8] * 7
NBLK = len(BLOCKS)

f8e4 = ml_dtypes.float8_e4m3
SCALE_BITS = 22
SCALE = float(2 ** SCALE_BITS)
INV_SCALE = float(2.0 ** (-SCALE_BITS))

_cached = {}

# --- permutation fast path ---------------------------------------------------
SWAP_MASK = (1 << 12) | (1 << 5)  # 4128: SWAP(0,7) on 13 qubits, bit-flipped
ROWS = 2 * BATCH // NCORES        # 16 rows per core (8 re + 8 im)


def _is_expected_perm(M):
    """True iff M is exactly the bit12<->bit5 column-swap permutation."""
    if M.shape != (N, N):
        return False
    idx = np.arange(N)
    differ = ((idx >> 12) & 1) != ((idx >> 5) & 1)
    swp = np.where(differ, idx ^ SWAP_MASK, idx)
    if not np.all(M[idx, swp] == 1.0):
        return False
    # the 8192 checked entries are exactly 1; 8192 nonzeros total => all
    # other entries are exactly 0, i.e. M is exactly this permutation
    return np.count_nonzero(M) == N


def _build_permute_program(loop_n=None, unroll=1, bufs=2, nout=1):
    """Permutation kernel: out[p, x, m, y, l] = x[p, y, m, x, l].

    DRAM/SBUF layout [128, 2, 8, 2, 32] f32: p = (row, col bits 11..9)
    (host pre-permuted), then free dims x=bit12, m=bits 8..6, y=bit5,
    l=bits 4..0.  loop_n!=None wraps `unroll` reps in a hardware For_i loop
    for slope timing (one NEFF, loop_n iterations); `bufs` is the SBUF
    double-buffer depth and `nout` the number of rotating DRAM output
    buffers (>1 breaks the benchmark loop's artificial store WAW chain --
    each rep still writes a full output).
    """
    import concourse.mybir as mybir
    import concourse.tile as tile
    from concourse import bacc

    f32 = mybir.dt.float32
    nc = bacc.Bacc("TRN2", target_bir_lowering=False, debug=False)
    x_d = nc.declare_dram_parameter("x", [128, 2, 8, 2, 32], f32, isOutput=False)
    out_shape = [128, 2, 8, 2, 32] if nout == 1 else [nout, 128, 2, 8, 2, 32]
    out_d = nc.declare_dram_parameter("out", out_shape, f32, isOutput=True)

    with tile.TileContext(nc) as tc:
        with tc.tile_pool(name="io", bufs=bufs) as iop:

            def rep(u):
                od = out_d if nout == 1 else out_d[u % nout]
                in_sb = iop.tile([128, 2, 8, 2, 32], f32, name="in_sb")
                out_sb = iop.tile([128, 2, 8, 2, 32], f32, name="out_sb")
                nc.sync.dma_start(in_sb[:], x_d[:])
                # swap bit12 (x) and bit5 (y): one x-half per engine so the
                # two copies run concurrently on DVE and ACT
                nc.vector.tensor_copy(
                    out_sb[:, 0], in_sb[:, :, :, 0, :].transpose([0, 2, 1, 3])
                )
                nc.scalar.copy(
                    out_sb[:, 1], in_sb[:, :, :, 1, :].transpose([0, 2, 1, 3])
                )
                nc.scalar.dma_start(od[:], out_sb[:])

            if loop_n is None:
                for u in range(unroll):
                    rep(u)
            else:
                with tc.For_i(0, loop_n):
                    for u in range(unroll):
                        rep(u)
    nc.compile()
    return nc


def _prep_perm_inputs(state_re, state_im):
    """Per-core [128, 2, 8, 2, 32] arrays: p=(row, col bits 11..9), then
    (bit12, bits 8..6, bit5, bits 4..0)."""
    maps = []
    rpc = ROWS // 2  # 8 batch rows per core
    for c in range(NCORES):
        rows = np.concatenate(
            [state_re[c * rpc:(c + 1) * rpc], state_im[c * rpc:(c + 1) * rpc]],
            axis=0,
        )  # [16, 8192]
        v = rows.reshape(ROWS, 2, 8, 512).transpose(0, 2, 1, 3)  # r, m53, x, rest
        maps.append({"x": np.ascontiguousarray(v).reshape(128, 2, 8, 2, 32)})
    return maps


def _post_perm(results):
    re_parts, im_parts = [], []
    rpc = ROWS // 2
    for c in range(NCORES):
        o = np.asarray(results[c]["out"]).reshape(ROWS, 8, 2, 512)
        o = o.transpose(0, 2, 1, 3).reshape(ROWS, N)
        re_parts.append(o[:rpc])
        im_parts.append(o[rpc:])
    out_re = np.concatenate(re_parts, axis=0)
    out_im = np.concatenate(im_parts, axis=0)
    return (out_re + 1j * out_im).astype(np.complex64)


# --- dense matmul fallback ---------------------------------------------------
def _fp8_exact(M):
    # cheap exactness check: fp8e4m3 round-trips M losslessly?
    sample = M[:: 64, :: 64]
    if not np.array_equal(sample.astype(f8e4).astype(np.float32), sample):
        return False
    return np.array_equal(M.astype(f8e4).astype(np.float32), M)


def _build_matmul_program(reps=1, serialize=False, m_dt="fp8"):
    # reps>1 repeats the whole pipeline inside one NEFF (for benchmarking);
    # serialize adds an all-engine barrier between reps so the per-rep slope
    # approximates a single-shot kernel execution.
    import concourse.mybir as mybir
    import concourse.tile as tile
    from concourse import bacc

    mdt = {"fp8": mybir.dt.float8e4, "bf16": mybir.dt.bfloat16}[m_dt]
    nc = bacc.Bacc("TRN2", target_bir_lowering=False, debug=False)
    st_d = nc.declare_dram_parameter("st", [P, KT, 256], mybir.dt.float16, isOutput=False)
    m_d = nc.declare_dram_parameter("m", [P, KT, NCH, 512], mdt, isOutput=False)
    out_d = nc.declare_dram_parameter("out", [P, COLS], mybir.dt.float32, isOutput=True)

    with tile.TileContext(nc) as tc:
        with (
            tc.tile_pool(name="stp", bufs=1) as stp,
            tc.tile_pool(name="mp", bufs=4) as mp,
            tc.tile_pool(name="op", bufs=1) as op,
            tc.tile_pool(name="ps", bufs=1, space="PSUM") as ps,
        ):
            st_sb = stp.tile([P, KT, 256], mybir.dt.float16)
            # split the state load so the first matmuls aren't gated on 4MB
            k0 = 0
            for nb in BLOCKS:
                nc.sync.dma_start(st_sb[:, k0:k0 + nb, :], st_d[:, k0:k0 + nb, :])
                k0 += nb
            # dummy matmuls on a zeroed scratch tile run during the initial
            # DMA wait and release the PE HAM clock throttle (1.2 -> 2.4 GHz)
            # before the real matmuls start (measured ~5us single-shot win)
            wsb = stp.tile([P, 128], mybir.dt.float16, name="wsb")
            nc.vector.memset(wsb[:], 0.0)
            wps = ps.tile([P, 128], mybir.dt.float32, name="wps")
            for _rep in range(reps):
                if serialize and reps > 1:
                    tc.strict_bb_all_engine_barrier()
                for _ in range(40):
                    nc.tensor.matmul(wps[:], wsb[:], wsb[:], start=True, stop=True)
                out_sb = op.tile([P, COLS], mybir.dt.float32, name="out_sb")
                ps_hi = [
                    ps.tile([P, 512], mybir.dt.float32, name=f"ps_hi{i}")
                    for i in range(NCH)
                ]
                ps_lo = [
                    ps.tile([P, 512], mybir.dt.float32, name=f"ps_lo{i}")
                    for i in range(NCH)
                ]
                k0 = 0
                for nb in BLOCKS:
                    m_sb = mp.tile([P, KBLK, NCH, 512], mdt, name="m_sb")
                    nc.sync.dma_start(m_sb[:, :nb], m_d[:, k0:k0 + nb, :, :])
                    for kj in range(nb):
                        ko = k0 + kj
                        # pass-major order: the stationary operand (hi or lo
                        # state tile) is reused across both n-chunks, halving
                        # LDWEIGHTS traffic vs alternating hi/lo per chunk
                        for pss, c0 in ((ps_hi, 0), (ps_lo, 128)):
                            for nch in range(NCH):
                                nc.tensor.matmul(
                                    pss[nch][:],
                                    st_sb[:, ko, c0:c0 + 128],
                                    m_sb[:, kj, nch, :],
                                    start=(ko == 0),
                                    stop=(ko == KT - 1),
                                )
                    k0 += nb
                for nch in range(NCH):
                    sl = slice(nch * 512, (nch + 1) * 512)
                    nc.vector.tensor_scalar_mul(out_sb[:, sl], ps_lo[nch][:], INV_SCALE)
                    nc.vector.tensor_add(out_sb[:, sl], out_sb[:, sl], ps_hi[nch][:])
                nc.sync.dma_start(out_d[:], out_sb[:])
    nc.compile()
    return nc


def _get_program(key, builder, **kw):
    if key not in _cached:
        _cached[key] = builder(**kw)
    return _cached[key]


def _prep_inputs(state_re, state_im, M, m_dt="fp8"):
    # Stationary layout: [8192, 256] fp16 where cols 0:64 re_hi, 64:128 im_hi,
    # 128:192 re_lo*2^22, 192:256 im_lo*2^22; tiled to [128 part, 64 kt, 256].
    S = np.empty((N, P), dtype=np.float32)
    S[:, :BATCH] = state_re.T
    S[:, BATCH:] = state_im.T
    hi = S.astype(np.float16)
    lo = ((S - hi.astype(np.float32)) * SCALE).astype(np.float16)
    stall = np.concatenate([hi, lo], axis=1)  # [8192, 256] fp16
    st_tiled = np.ascontiguousarray(
        stall.reshape(KT, P, 256).transpose(1, 0, 2)
    )  # [128, 64, 256]

    Mb = M.astype(f8e4 if m_dt == "fp8" else ml_dtypes.bfloat16)
    m_tiles = []
    for c in range(NCORES):
        shard = Mb[:, c * COLS:(c + 1) * COLS]
        m_tiles.append(
            np.ascontiguousarray(
                shard.reshape(KT, P, NCH, 512).transpose(1, 0, 2, 3)
            )
        )  # [128, 64, 2, 512]
    return st_tiled, m_tiles


def run_on_hw(state_re, state_im, M, trace=False):
    from concourse.bass_utils import run_bass_kernel_spmd

    state_re = np.asarray(state_re, dtype=np.float32)
    state_im = np.asarray(state_im, dtype=np.float32)
    M = np.asarray(M, dtype=np.float32)

    if state_re.shape == (BATCH, N) and _is_expected_perm(M):
        # fast path: M is exactly the SWAP permutation -> pure data movement
        nc = _get_program("perm", _build_permute_program)
        in_maps = _prep_perm_inputs(state_re, state_im)
        res = run_bass_kernel_spmd(
            nc, in_maps, list(range(NCORES)), trace=trace,
            trace_cores=list(range(NCORES)) if trace else None,
        )
        return _post_perm(res.results), res

    # fallback: dense matmul.  fp8e4m3 storage of M is exact only for values
    # with <=4 significand bits; fall back to bf16 if fp8 would round.
    m_dt = "fp8" if _fp8_exact(M) else "bf16"
    nc = _get_program(f"nc_{m_dt}", _build_matmul_program, m_dt=m_dt)
    st_tiled, m_tiles = _prep_inputs(state_re, state_im, M, m_dt)
    in_maps = [{"st": st_tiled, "m": m_tiles[c]} for c in range(NCORES)]
    res = run_bass_kernel_spmd(
        nc, in_maps, list(range(NCORES)), trace=trace,
        trace_cores=list(range(NCORES)) if trace else None,
    )
    full = np.concatenate([res.results[c]["out"] for c in range(NCORES)], axis=1)
    out = (full[:BATCH] + 1j * full[BATCH:]).astype(np.complex64)
    return out, res


def kernel(state_re, state_im, M):
    out, _ = run_on_hw(state_re, state_im, M, trace=False)
    return out

